# revision 2
# baseline (speedup 1.0000x reference)
"""Trainium2 Bass kernel for nn_KnowledgeIntegrationLoss (optimized v2).

Reference semantics:
    x = [new_knowledge; existing_knowledge]            # [11, 8192]
    E = MLP_encoder(x)                                 # [11, 128]
    geo_j = ||E_0 - E_j||, cos_j = <E_0,E_j>/(max(|E_0|,eps)*max(|E_j|,eps))
    avg = mean_{j=1..10}(geo_j - cos_j)
    q = MLP_integrator([E_0; math_metrics])            # [1]
    out = avg + exp(-q)                                # scalar

Two-launch structure (collectives cost 70+us on this runtime; the
per-launch floor is ~14us):
  Phase 1 (SPMD x8): column-shard W1; core j computes
      h1T_j = ReLU(x @ W1[:, 64j:64j+64] + b1[...]).T  -> [64, 11] bf16
    Matmuls run in bf16 (or fp8e4m3 DoubleRow: 2 k-tiles/instr, W1
    pre-scaled by 64 to avoid fp8 subnormals, 1/64 folded into the
    ReLU copy). Bias rides as an extra contraction tile.
  Host: pure concat/layout shuffle (zero FLOPs).
  Phase 2 (1 core): layers 2..3 + loss tail. Loss uses the Gram matrix
      G = E^T E (one matmul): geo2_j = G_jj + G_00 - 2 G_j0,
      cos_j = G_j0 * exp(-0.5 ln(G_00 G_jj)). sqrt/rsqrt go through
      ln+exp so only ONE activation table set loads (prefetched early).
"""

import numpy as np

import concourse.bass as bass
import concourse.mybir as mybir
import concourse.tile as tile
from concourse import bacc
from concourse import bass_utils

F32 = mybir.dt.float32
BF16 = mybir.dt.bfloat16
FP8 = mybir.dt.float8e4
N_CORES = 8
KDIM = 8192
ALU = mybir.AluOpType
AF = mybir.ActivationFunctionType

NP_BF16 = mybir.dt.np(BF16)
NP_FP8 = mybir.dt.np(FP8)

PHASE1_MODE = "fp8dr"  # "bf16" | "fp8dr"
W1_SCALE = 64.0  # fp8 only: W1 pre-scale (power of 2), undone in the ReLU
NEWTON_ITERS = 1  # rsqrt refinement; 1 iter of quake-seed -> ~1.8e-3 rel
P2_FP8 = True  # phase 2: h1t+W2 in fp8e4m3, layer 2 as DoubleRow
W2_SCALE = 32.0  # fp8 only: W2 pre-scale (power of 2), undone in the ReLU

# ---------------------------------------------------------------------------
# host-side layout helpers (pure reshapes/transposes/dtype casts, no FLOPs)
# ---------------------------------------------------------------------------


def _kmajor_image(a, p=128):
    """[K, M] (K = c*p + part) -> [p, (K//p)*M]: img[part, c*M+m] = a[c*p+part, m]."""
    K, M = a.shape
    n = K // p
    return np.ascontiguousarray(a.reshape(n, p, M).transpose(1, 0, 2).reshape(p, n * M))


# ---------------------------------------------------------------------------
# phase 1: per-core first-layer column shard
# ---------------------------------------------------------------------------

# 64 real k-tiles + bias tile (+1 zero pad tile in fp8dr mode)
NKT = 66 if PHASE1_MODE == "fp8dr" else 65
P1DT = FP8 if PHASE1_MODE == "fp8dr" else BF16
XCOLS = NKT * 11
WCOLS = NKT * 64


def build_phase1():
    nc = bacc.Bacc("TRN2", target_bir_lowering=False, debug=False,
                   num_devices=N_CORES)

    x_img = nc.dram_tensor("x_img", [128, XCOLS], P1DT, kind="ExternalInput")
    w_img = nc.dram_tensor("w_img", [128, WCOLS], P1DT, kind="ExternalInput")
    h1t_out = nc.dram_tensor("h1t_out", [64, 11], BF16, kind="ExternalOutput")

    with tile.TileContext(nc) as tc:
        with (
            tc.tile_pool(name="sbuf", bufs=1) as sb,
            tc.tile_pool(name="psum", bufs=1, space="PSUM") as ps,
        ):
            # x image on the scalar ring first (it gates every matmul)
            xsb = sb.tile([128, XCOLS], P1DT, tag="xsb")
            nc.scalar.dma_start(xsb[:], x_img[:])

            # W1 chunks in consumption order on independent rings
            chunk_plan = [
                (0, 24, nc.sync),
                (24, 48, nc.gpsimd),
                (48, NKT, nc.scalar),  # queued behind x on the scalar ring
            ]
            wsb = sb.tile([128, WCOLS], P1DT, tag="wsb")
            for a, b, eng in chunk_plan:
                eng.dma_start(wsb[:, 64 * a:64 * b], w_img[:, 64 * a:64 * b])

            # PE warm-up: tiny junk matmuls keep the HAM activity window
            # busy until x/W land, so real matmuls run at full clock.
            junk = sb.tile([128, 16], BF16, tag="junk")
            nc.vector.memset(junk[:], 0.0)
            jps = ps.tile([11, 1], F32, tag="jps")
            for _ in range(24):
                nc.tensor.matmul(jps[:, :], junk[:, 0:11], junk[:, 0:1],
                                 start=True, stop=True)

            psum1 = ps.tile([64, 11], F32, tag="psum1")
            if PHASE1_MODE == "fp8dr":
                DR = mybir.MatmulPerfMode.DoubleRow
                for i in range(NKT // 2):
                    nc.tensor.matmul(
                        psum1[:, :],
                        wsb[:, 128 * i:128 * (i + 1)].rearrange(
                            "p (two f) -> p two f", two=2),
                        xsb[:, 22 * i:22 * (i + 1)].rearrange(
                            "p (two f) -> p two f", two=2),
                        start=(i == 0),
                        stop=(i == NKT // 2 - 1),
                        perf_mode=DR,
                    )
            else:
                for i in range(NKT):
                    nc.tensor.matmul(
                        psum1[:, :],
                        wsb[:, 64 * i:64 * (i + 1)],
                        xsb[:, 11 * i:11 * (i + 1)],
                        start=(i == 0),
                        stop=(i == NKT - 1),
                    )

            h1b = sb.tile([64, 11], BF16, tag="h1b")
            unscale = 1.0 / W1_SCALE if PHASE1_MODE == "fp8dr" else 1.0
            # h1t block = max(psum * unscale, 0)
            nc.vector.tensor_scalar(h1b[:], psum1[:], unscale, 0.0,
                                    ALU.mult, ALU.max)
            nc.sync.dma_start(h1t_out[:], h1b[:])
    nc.compile()
    return nc


def phase1_inputs(x, W1, b1):
    """Per-core input maps for phase 1. x: [11, 8192] f32."""
    scale = W1_SCALE if PHASE1_MODE == "fp8dr" else 1.0
    npdt = NP_FP8 if PHASE1_MODE == "fp8dr" else NP_BF16

    # x image [128, NKT*11]: tiles 0..63 = k-major x^T, tile 64 = e vector
    # (partition 0 = 1, bias rider), tile 65 (fp8dr only) = zeros.
    xi = np.zeros((128, NKT, 11), np.float32)
    xi[:, 0:64, :] = x.T.reshape(64, 128, 11).transpose(1, 0, 2)
    xi[0, 64, :] = 1.0
    xi = xi.reshape(128, XCOLS).astype(npdt)

    maps = []
    for j in range(N_CORES):
        wj = np.zeros((128, NKT, 64), np.float32)
        w1j = scale * W1[:, 64 * j:64 * (j + 1)]          # [8192, 64]
        wj[:, 0:64, :] = w1j.reshape(64, 128, 64).transpose(1, 0, 2)
        wj[0, 64, :] = scale * b1[64 * j:64 * (j + 1)]
        maps.append({
            "x_img": xi,
            "w_img": wj.reshape(128, WCOLS).astype(npdt),
        })
    return maps


# ---------------------------------------------------------------------------
# phase 2: layers 2..3 + loss tail, single core
# ---------------------------------------------------------------------------

if P2_FP8:
    # imgA (sync ring, fp8): h1t image [128,44] | W2 DoubleRow-ordered
    # [128,1024] (h-major: per h, per tile-pair, [W2(2p,h)|W2(2p+1,h)]),
    # pre-scaled by W2_SCALE (fp8 subnormal dodge; undone via b2*scale +
    # W3/scale).
    A_DT = FP8
    _A_SHAPES = [("h1t", 128, 44), ("w2dr", 128, 1024)]
    _B_SHAPES = [
        ("w3", 128, 256), ("wi1a", 128, 64), ("wi1b7", 7, 64),
        ("mm6e", 7, 1), ("wi2e", 65, 32), ("wi3e", 33, 1),
    ]
else:
    A_DT = BF16
    _A_SHAPES = [
        ("h1t", 128, 44), ("w2a", 128, 512), ("wi1a", 128, 64),
        ("wi1b7", 7, 64), ("mm6e", 7, 1),
    ]
    _B_SHAPES = [
        ("w2b", 128, 512), ("w3", 128, 256), ("wi2e", 65, 32),
        ("wi3e", 33, 1),
    ]
A_OFF = {}
_c = 0
for _n, _p, _f in _A_SHAPES:
    A_OFF[_n] = _c
    _c += _f
IMGA_COLS = _c
B_OFF = {}
_c = 0
for _n, _p, _f in _B_SHAPES:
    B_OFF[_n] = _c
    _c += _f
IMGB_COLS = _c
# imgC (f32 consts, sync ring first): b2 [128,2] | b3c [128,1] |
# Imask [11,11] | wmean [11,1] | ones1 [1,11]
_C_SHAPES = [
    ("b2", 128, 2), ("b3c", 128, 1), ("imask", 11, 11), ("wmean", 11, 1),
    ("ones1", 1, 11),
]
C_OFF = {}
_c = 0
for _n, _p, _f in _C_SHAPES:
    C_OFF[_n] = _c
    _c += _f
IMGC_COLS = _c


def build_phase2():
    nc = bacc.Bacc("TRN2", target_bir_lowering=False, debug=False, num_devices=1)

    imgA = nc.dram_tensor("imgA", [128, IMGA_COLS], A_DT, kind="ExternalInput")
    imgB = nc.dram_tensor("imgB", [128, IMGB_COLS], BF16, kind="ExternalInput")
    imgC = nc.dram_tensor("imgC", [128, IMGC_COLS], F32, kind="ExternalInput")
    out = nc.dram_tensor("out", [1, 1], F32, kind="ExternalOutput")

    with tile.TileContext(nc) as tc:
        with (
            tc.tile_pool(name="sbuf", bufs=1) as sb,
            tc.tile_pool(name="psum", bufs=1, space="PSUM") as ps,
        ):
            # two DMAs per ring, split at matmul-consumption boundaries so
            # the first layer-2 matmuls start as soon as possible
            asb = sb.tile([128, IMGA_COLS], A_DT, tag="asb")
            bsb = sb.tile([128, IMGB_COLS], BF16, tag="bsb")
            ASPLIT = 44 + 512
            if P2_FP8:
                # fine split in consumption order: W2 h0-pair0 gates the
                # first matmul (small first chunks on every ring)
                nc.sync.dma_start(asb[:, 0:300], imgA[:, 0:300])
                nc.scalar.dma_start(bsb[:], imgB[:])
                nc.gpsimd.dma_start(asb[:, 300:ASPLIT], imgA[:, 300:ASPLIT])
                nc.sync.dma_start(asb[:, ASPLIT:ASPLIT + 256],
                                  imgA[:, ASPLIT:ASPLIT + 256])
                nc.scalar.dma_start(asb[:, ASPLIT + 256:IMGA_COLS],
                                    imgA[:, ASPLIT + 256:IMGA_COLS])
            else:
                nc.sync.dma_start(asb[:, 0:300], imgA[:, 0:300])
                nc.scalar.dma_start(bsb[:, 0:256], imgB[:, 0:256])
                nc.sync.dma_start(asb[:, 300:IMGA_COLS], imgA[:, 300:IMGA_COLS])
                nc.scalar.dma_start(bsb[:, 256:IMGB_COLS],
                                    imgB[:, 256:IMGB_COLS])
            csb = sb.tile([128, IMGC_COLS], F32, tag="csb")
            nc.gpsimd.dma_start(csb[:], imgC[:])

            def cs(name, p_, f_):
                c0 = C_OFF[name]
                return csb[0:p_, c0:c0 + f_]

            def bs(name, p_, f_):
                c0 = B_OFF[name]
                return bsb[0:p_, c0:c0 + f_]

            def as_(name, p_, f_):
                c0 = A_OFF[name]
                return asb[0:p_, c0:c0 + f_]

            b2sb = cs("b2", 128, 2)
            b3c = cs("b3c", 128, 1)
            imask = cs("imask", 11, 11)
            wmsb = cs("wmean", 11, 1)
            onsb = cs("ones1", 1, 11)

            w3sb = bs("w3", 128, 256)
            if P2_FP8:
                wi1asb = bs("wi1a", 128, 64)
                wi1b7 = bs("wi1b7", 7, 64)
                mm6e = bs("mm6e", 7, 1)
            else:
                wi1asb = as_("wi1a", 128, 64)
                wi1b7 = as_("wi1b7", 7, 64)
                mm6e = as_("mm6e", 7, 1)
            wi2e = bs("wi2e", 65, 32)
            wi3e = bs("wi3e", 33, 1)

            h1sb = asb[:, 0:44]

            def w2l(t, h):  # lhsT [128,128]: W2[128t+p, 128h+m]
                if t < 2:
                    return asb[:, 44 + 256 * t + 128 * h:44 + 256 * t + 128 * (h + 1)]
                return bsb[:, 256 * (t - 2) + 128 * h:256 * (t - 2) + 128 * (h + 1)]

            # hidden vectors with trailing 1.0 partition (bias K-extension)
            i1r = sb.tile([65, 1], BF16, tag="i1r")
            nc.vector.memset(i1r[64:65, :], 1.0)
            i2r = sb.tile([33, 1], BF16, tag="i2r")
            nc.vector.memset(i2r[32:33, :], 1.0)

            # exp activation-table prefetch, GpSimd extended-ucode library
            # prefetch (for the mid-chain partition_broadcast), and PE
            # warm-up -- all overlapped with the input DMA wait
            junk = sb.tile([128, 16], BF16, tag="junk")
            nc.vector.memset(junk[:], 0.0)
            jf = sb.tile([1, 2], F32, tag="jf")
            nc.vector.memset(jf[:], 1.0)
            ones128 = sb.tile([128, 11], F32, tag="ones128")
            nc.vector.memset(ones128[:], 1.0)
            nc.scalar.activation(jf[0:1, 1:2], jf[0:1, 0:1], AF.Exp)
            jps = ps.tile([11, 1], F32, tag="jps")
            for _ in range(12):
                nc.tensor.matmul(jps[:, :], junk[:, 0:11], junk[:, 0:1],
                                 start=True, stop=True)

            # ---- layer 2 direct to h2T: psum [128,11] per half h
            # (fp8 path: DoubleRow over k-tile pairs, W2 pre-scaled by
            # W2_SCALE; h2t carries the scale, undone in W3)
            h2t = sb.tile([128, 22], BF16, tag="h2t")
            for h in range(2):
                p2t = ps.tile([128, 11], F32, tag="pA", bufs=3)
                if P2_FP8:
                    DR2 = mybir.MatmulPerfMode.DoubleRow
                    for p in range(2):
                        c0 = 44 + 512 * h + 256 * p
                        nc.tensor.matmul(
                            p2t[:, :],
                            asb[:, c0:c0 + 256].rearrange(
                                "q (two f) -> q two f", two=2),
                            h1sb[:, 22 * p:22 * (p + 1)].rearrange(
                                "q (two f) -> q two f", two=2),
                            start=(p == 0), stop=(p == 1),
                            perf_mode=DR2,
                        )
                else:
                    for t in range(4):
                        nc.tensor.matmul(
                            p2t[:, :], w2l(t, h), h1sb[:, 11 * t:11 * (t + 1)],
                            start=(t == 0), stop=(t == 3),
                        )
                # relu(x + b2') on DVE, cast to bf16 (b2' pre-scaled host-side)
                nc.vector.tensor_scalar(h2t[:, 11 * h:11 * (h + 1)], p2t[:],
                                        b2sb[:, h:h + 1], 0.0, ALU.add, ALU.max)

            # ---- layer 3 col form: E^T [128, 11] (all 11 encodings)
            psET = ps.tile([128, 11], F32, tag="pA", bufs=3)
            for h in range(2):
                nc.tensor.matmul(
                    psET[:, :], w3sb[:, 128 * h:128 * (h + 1)],
                    h2t[:, 11 * h:11 * (h + 1)],
                    start=(h == 0), stop=(h == 1),
                )
            ETsb = sb.tile([128, 11], F32, tag="ETsb")
            # ET = psET + b3 (per-partition bias broadcast along free dim)
            nc.vector.tensor_scalar(ETsb[:], psET[:], b3c, 0.0,
                                    ALU.add, ALU.bypass)
            # E0 replicated along the free dim (feeds the G00 broadcast
            # matmul, which then overlaps the gram + diag)
            E0rep = sb.tile([128, 11], F32, tag="E0rep")
            nc.vector.tensor_scalar(E0rep[:], ones128, ETsb[0:128, 0:1], 0.0,
                                    ALU.mult, ALU.bypass)
            newT16 = sb.tile([128, 1], BF16, tag="newT16")
            nc.vector.tensor_copy(newT16[:], ETsb[:, 0:1])

            # ---- Gram matrix G = ET.T @ ET -> [11, 11] (f32 matmul)
            G = ps.tile([11, 11], F32, tag="pG")
            nc.tensor.matmul(G[:, :], ETsb[:], ETsb[:], start=True, stop=True)
            # ---- G00 = |E0|^2 on all 11 partitions, without waiting for
            # the diag: bc00 = E0rep.T @ E0
            bc00 = ps.tile([11, 1], F32, tag="pB")
            nc.tensor.matmul(bc00[:, :], E0rep[:], ETsb[:, 0:1],
                             start=True, stop=True)

            # ---- integrator MLP on [E_0; math_metrics] (parallel branch)
            i1c = ps.tile([64, 1], F32, tag="pC", bufs=2)
            nc.tensor.matmul(i1c[:, :], wi1asb, newT16[:, 0:1],
                             start=True, stop=False)
            nc.tensor.matmul(i1c[:, :], wi1b7, mm6e, start=False, stop=True)
            # integrator relus on the Scalar engine (relu is in every act
            # table set -> no extra table load; keeps the DVE chain clear)
            nc.scalar.activation(i1r[0:64, :], i1c[:, :], AF.Relu)

            # ---- diag(G) -> n2 [11,1] (DVE); col 0 of G = dvec (in place)
            scrG = sb.tile([11, 11], F32, tag="scrG")
            n2 = sb.tile([11, 1], F32, tag="n2")
            nc.vector.scalar_tensor_tensor(
                out=scrG[:], in0=G[:], scalar=1.0, in1=imask,
                op0=ALU.mult, op1=ALU.mult, accum_out=n2[:])
            dvec = G[:, 0:1]

            i2c = ps.tile([32, 1], F32, tag="pC", bufs=2)
            nc.tensor.matmul(i2c[:, :], wi2e, i1r[:, 0:1], start=True, stop=True)
            nc.scalar.activation(i2r[0:32, :], i2c[:, :], AF.Relu)

            # ---- ge [11,2]: col0 = clamp(g2), col1 = G00*n2
            ge = sb.tile([11, 2], F32, tag="ge")
            s12 = sb.tile([11, 1], F32, tag="s12")
            nc.vector.tensor_add(s12[:], n2[:], bc00[:])
            nc.vector.scalar_tensor_tensor(
                out=ge[:, 0:1], in0=dvec, scalar=-2.0, in1=s12[:],
                op0=ALU.mult, op1=ALU.add)
            nc.vector.tensor_mul(ge[:, 1:2], n2[:], bc00[:])
            nc.vector.tensor_scalar_max(ge[:, 0:1], ge[:, 0:1], 1e-12)

            qp = ps.tile([1, 1], F32, tag="pC", bufs=2)
            nc.tensor.matmul(qp[:, :], wi3e, i2r[:, 0:1], start=True, stop=True)
            il = sb.tile([1, 1], F32, tag="il")
            nc.scalar.activation(il[:], qp[:], AF.Exp, scale=-1.0)

            # ---- r = rsqrt([g2 | G00*n2]): quake seed + fused Newton
            # iters, DVE only (no second activation-table load).
            # geo = g2 * rsqrt(g2); rden = rsqrt(G00*n2) = 1/(|E0||Ej|).
            I32 = mybir.dt.int32
            seedb = sb.tile([11, 2], I32, tag="seedb")
            nc.vector.tensor_scalar(seedb[:], ge[:].bitcast(I32), 1, None,
                                    ALU.arith_shift_right)
            # r0 = bitcast(0x5f3759df - (bits >> 1))
            nc.vector.tensor_scalar(seedb[:], seedb[:], -1, 0x5F3759DF,
                                    ALU.mult, ALU.add)
            r = sb.tile([11, 2], F32, tag="rsq")
            rr = sb.tile([11, 2], F32, tag="rr")
            rcur = seedb[:].bitcast(F32)
            for _ in range(NEWTON_ITERS):
                nc.vector.tensor_mul(rr[:], rcur, rcur)
                # t2 = (-0.5*r^2) * ge
                nc.vector.scalar_tensor_tensor(
                    out=rr[:], in0=rr[:], scalar=-0.5, in1=ge[:],
                    op0=ALU.mult, op1=ALU.mult)
                # r' = (t2 + 1.5) * r
                nc.vector.scalar_tensor_tensor(
                    out=r[:], in0=rr[:], scalar=1.5, in1=rcur,
                    op0=ALU.add, op1=ALU.mult)
                rcur = r[:]
            geo = sb.tile([11, 1], F32, tag="geo")
            nc.vector.tensor_mul(geo[:], ge[:, 0:1], r[:, 0:1])

            # negscore = cos - geo = dvec*rden - geo
            negscore = sb.tile([11, 1], F32, tag="negscore")
            nc.vector.scalar_tensor_tensor(
                out=negscore[:], in0=dvec, scalar=r[0:11, 1:2],
                in1=geo[:], op0=ALU.mult, op1=ALU.subtract)

            # mean over rows 1..10 with negated weights (wmean = [0, -0.1 x10])
            meanp = ps.tile([1, 1], F32, tag="pC", bufs=2)
            nc.tensor.matmul(meanp[:, :], negscore[:, 0:1], wmsb,
                             start=True, stop=True)

            total = sb.tile([1, 1], F32, tag="total")
            nc.vector.tensor_add(total[:], il[:], meanp[:])
            nc.sync.dma_start(out[:], total[:], single_packet=True)
    nc.compile()
    return nc


def phase2_inputs(h1t_full, W2, b2, W3, b3, Wi1, bi1, Wi2, bi2, Wi3, bi3,
                  math_metrics):
    """h1t_full: [512, 11] f32-ish = concat of per-core [64, 11] outputs."""
    ints = {
        "wi1a": Wi1[:128],
        "wi1b7": np.concatenate([Wi1[128:], bi1.reshape(1, 64)], axis=0),
        "mm6e": np.concatenate([math_metrics.reshape(6, 1),
                                np.ones((1, 1), np.float32)], axis=0),
        "wi2e": np.concatenate([Wi2, bi2.reshape(1, 32)], axis=0),
        "wi3e": np.concatenate([Wi3, bi3.reshape(1, 1)], axis=0),
    }
    if P2_FP8:
        # DoubleRow-ordered W2, pre-scaled; scale undone via W3/scale and
        # pre-scaled b2 (h2t carries the scale)
        w2s = (W2_SCALE * W2).astype(np.float32)
        w2dr = np.zeros((128, 1024), np.float32)
        for h in range(2):
            for pair in range(2):
                for sub in range(2):
                    c = 512 * h + 256 * pair + 128 * sub
                    kt = 2 * pair + sub
                    w2dr[:, c:c + 128] = w2s[128 * kt:128 * (kt + 1),
                                             128 * h:128 * (h + 1)]
        avals = {"h1t": _kmajor_image(h1t_full.astype(np.float32)),
                 "w2dr": w2dr}
        bvals = {"w3": _kmajor_image(W3) / W2_SCALE, **ints}
        b2c = (W2_SCALE * b2).reshape(2, 128).T
        a_np = NP_FP8
    else:
        w2img = _kmajor_image(W2.astype(np.float32))  # [128, 1024]
        avals = {
            "h1t": _kmajor_image(h1t_full.astype(np.float32)),
            "w2a": w2img[:, 0:512],
            "wi1a": ints["wi1a"], "wi1b7": ints["wi1b7"],
            "mm6e": ints["mm6e"],
        }
        bvals = {
            "w2b": w2img[:, 512:1024],
            "w3": _kmajor_image(W3),
            "wi2e": ints["wi2e"], "wi3e": ints["wi3e"],
        }
        b2c = b2.reshape(2, 128).T
        a_np = NP_BF16

    imgA = np.zeros((128, IMGA_COLS), np.float32)
    for name, p, f in _A_SHAPES:
        v = np.asarray(avals[name], np.float32)
        assert v.shape == (p, f), (name, v.shape, (p, f))
        imgA[:p, A_OFF[name]:A_OFF[name] + f] = v

    imgB = np.zeros((128, IMGB_COLS), np.float32)
    for name, p, f in _B_SHAPES:
        v = np.asarray(bvals[name], np.float32)
        assert v.shape == (p, f), (name, v.shape, (p, f))
        imgB[:p, B_OFF[name]:B_OFF[name] + f] = v

    wm = np.zeros((11, 1), np.float32)
    wm[1:, 0] = -0.1
    cvals = {
        "b2": b2c,
        "b3c": b3.reshape(128, 1),
        "imask": np.eye(11, dtype=np.float32),
        "wmean": wm,
        "ones1": np.ones((1, 11), np.float32),
    }
    imgC = np.zeros((128, IMGC_COLS), np.float32)
    for name, p, f in _C_SHAPES:
        v = np.asarray(cvals[name], np.float32)
        assert v.shape == (p, f), (name, v.shape, (p, f))
        imgC[:p, C_OFF[name]:C_OFF[name] + f] = v
    return {"imgA": imgA.astype(a_np), "imgB": imgB.astype(NP_BF16),
            "imgC": imgC}


# ---------------------------------------------------------------------------
# entry point
# ---------------------------------------------------------------------------

_NC1 = None
_NC2 = None


def _get_ncs():
    global _NC1, _NC2
    if _NC1 is None:
        _NC1 = build_phase1()
        _NC2 = build_phase2()
    return _NC1, _NC2


def kernel(new_knowledge, existing_knowledge, math_metrics,
           W1, b1, W2, b2, W3, b3, Wi1, bi1, Wi2, bi2, Wi3, bi3):
    args = [new_knowledge, existing_knowledge, math_metrics,
            W1, b1, W2, b2, W3, b3, Wi1, bi1, Wi2, bi2, Wi3, bi3]
    (new_knowledge, existing_knowledge, math_metrics,
     W1, b1, W2, b2, W3, b3, Wi1, bi1, Wi2, bi2, Wi3, bi3) = [
        np.asarray(a, np.float32) for a in args]

    nc1, nc2 = _get_ncs()

    x = np.concatenate([new_knowledge[None, :], existing_knowledge], axis=0)
    maps1 = phase1_inputs(x, W1, b1)
    res1 = bass_utils.run_bass_kernel_spmd(
        nc1, maps1, core_ids=list(range(N_CORES)))
    # pure gather: concat per-core transposed h1 blocks -> [512, 11]
    h1t_full = np.concatenate(
        [np.asarray(res1.results[j]["h1t_out"], np.float32)
         for j in range(N_CORES)], axis=0)

    maps2 = [phase2_inputs(h1t_full, W2, b2, W3, b3,
                           Wi1, bi1, Wi2, bi2, Wi3, bi3, math_metrics)]
    res2 = bass_utils.run_bass_kernel_spmd(nc2, maps2, core_ids=[0])
    return res2.results[0]["out"].reshape(()).astype(np.float32)


# revision 3
# speedup vs baseline: 1.0155x; 1.0155x over previous
"""Trainium2 Bass kernel for nn_KnowledgeIntegrationLoss (optimized v2).

Reference semantics:
    x = [new_knowledge; existing_knowledge]            # [11, 8192]
    E = MLP_encoder(x)                                 # [11, 128]
    geo_j = ||E_0 - E_j||, cos_j = <E_0,E_j>/(max(|E_0|,eps)*max(|E_j|,eps))
    avg = mean_{j=1..10}(geo_j - cos_j)
    q = MLP_integrator([E_0; math_metrics])            # [1]
    out = avg + exp(-q)                                # scalar

Two-launch structure (collectives cost 70+us on this runtime; the
per-launch floor is ~14us):
  Phase 1 (SPMD x8): column-shard W1; core j computes
      h1T_j = ReLU(x @ W1[:, 64j:64j+64] + b1[...]).T  -> [64, 11] bf16
    Matmuls run in bf16 (or fp8e4m3 DoubleRow: 2 k-tiles/instr, W1
    pre-scaled by 64 to avoid fp8 subnormals, 1/64 folded into the
    ReLU copy). Bias rides as an extra contraction tile.
  Host: pure concat/layout shuffle (zero FLOPs).
  Phase 2 (1 core): layers 2..3 + loss tail. Loss uses the Gram matrix
      G = E^T E (one matmul): geo2_j = G_jj + G_00 - 2 G_j0,
      cos_j = G_j0 * rsqrt(G_00 G_jj). rsqrt/sqrt run on the DVE via a
      quake-style bit-trick seed + one fused Newton iteration, so the
      only Scalar-engine activations are Exp/Relu (one table set,
      prefetched during the input DMA wait). Layer 2 runs in fp8e4m3
      DoubleRow (W2 pre-scaled by 32; undone via b2*32 and W3/32).
"""

import numpy as np

import concourse.bass as bass
import concourse.mybir as mybir
import concourse.tile as tile
from concourse import bacc
from concourse import bass_utils

F32 = mybir.dt.float32
BF16 = mybir.dt.bfloat16
FP8 = mybir.dt.float8e4
N_CORES = 8
KDIM = 8192
ALU = mybir.AluOpType
AF = mybir.ActivationFunctionType

NP_BF16 = mybir.dt.np(BF16)
NP_FP8 = mybir.dt.np(FP8)

PHASE1_MODE = "fp8dr"  # "bf16" | "fp8dr"
W1_SCALE = 64.0  # fp8 only: W1 pre-scale (power of 2), undone in the ReLU
NEWTON_ITERS = 1  # rsqrt refinement; 1 iter of quake-seed -> ~1.8e-3 rel
P2_FP8 = True  # phase 2: h1t+W2 in fp8e4m3, layer 2 as DoubleRow
W2_SCALE = 32.0  # fp8 only: W2 pre-scale (power of 2), undone in the ReLU

# ---------------------------------------------------------------------------
# host-side layout helpers (pure reshapes/transposes/dtype casts, no FLOPs)
# ---------------------------------------------------------------------------


def _kmajor_image(a, p=128):
    """[K, M] (K = c*p + part) -> [p, (K//p)*M]: img[part, c*M+m] = a[c*p+part, m]."""
    K, M = a.shape
    n = K // p
    return np.ascontiguousarray(a.reshape(n, p, M).transpose(1, 0, 2).reshape(p, n * M))


# ---------------------------------------------------------------------------
# phase 1: per-core first-layer column shard
# ---------------------------------------------------------------------------

# 64 real k-tiles + bias tile (+1 zero pad tile in fp8dr mode)
NKT = 66 if PHASE1_MODE == "fp8dr" else 65
P1DT = FP8 if PHASE1_MODE == "fp8dr" else BF16
XCOLS = NKT * 11
WCOLS = NKT * 64


def build_phase1():
    nc = bacc.Bacc("TRN2", target_bir_lowering=False, debug=False,
                   num_devices=N_CORES)

    x_img = nc.dram_tensor("x_img", [128, XCOLS], P1DT, kind="ExternalInput")
    w_img = nc.dram_tensor("w_img", [128, WCOLS], P1DT, kind="ExternalInput")
    h1t_out = nc.dram_tensor("h1t_out", [64, 11], BF16, kind="ExternalOutput")

    with tile.TileContext(nc) as tc:
        with (
            tc.tile_pool(name="sbuf", bufs=1) as sb,
            tc.tile_pool(name="psum", bufs=1, space="PSUM") as ps,
        ):
            # x image on the scalar ring first (it gates every matmul)
            xsb = sb.tile([128, XCOLS], P1DT, tag="xsb")
            nc.scalar.dma_start(xsb[:], x_img[:])

            # W1 chunks in consumption order on independent rings
            chunk_plan = [
                (0, 24, nc.sync),
                (24, 48, nc.gpsimd),
                (48, NKT, nc.scalar),  # queued behind x on the scalar ring
            ]
            wsb = sb.tile([128, WCOLS], P1DT, tag="wsb")
            for a, b, eng in chunk_plan:
                eng.dma_start(wsb[:, 64 * a:64 * b], w_img[:, 64 * a:64 * b])

            # PE warm-up: tiny junk matmuls keep the HAM activity window
            # busy until x/W land, so real matmuls run at full clock.
            junk = sb.tile([128, 16], BF16, tag="junk")
            nc.vector.memset(junk[:], 0.0)
            jps = ps.tile([11, 1], F32, tag="jps")
            for _ in range(24):
                nc.tensor.matmul(jps[:, :], junk[:, 0:11], junk[:, 0:1],
                                 start=True, stop=True)

            psum1 = ps.tile([64, 11], F32, tag="psum1")
            if PHASE1_MODE == "fp8dr":
                DR = mybir.MatmulPerfMode.DoubleRow
                for i in range(NKT // 2):
                    nc.tensor.matmul(
                        psum1[:, :],
                        wsb[:, 128 * i:128 * (i + 1)].rearrange(
                            "p (two f) -> p two f", two=2),
                        xsb[:, 22 * i:22 * (i + 1)].rearrange(
                            "p (two f) -> p two f", two=2),
                        start=(i == 0),
                        stop=(i == NKT // 2 - 1),
                        perf_mode=DR,
                    )
            else:
                for i in range(NKT):
                    nc.tensor.matmul(
                        psum1[:, :],
                        wsb[:, 64 * i:64 * (i + 1)],
                        xsb[:, 11 * i:11 * (i + 1)],
                        start=(i == 0),
                        stop=(i == NKT - 1),
                    )

            h1b = sb.tile([64, 11], BF16, tag="h1b")
            unscale = 1.0 / W1_SCALE if PHASE1_MODE == "fp8dr" else 1.0
            # h1t block = max(psum * unscale, 0)
            nc.vector.tensor_scalar(h1b[:], psum1[:], unscale, 0.0,
                                    ALU.mult, ALU.max)
            nc.sync.dma_start(h1t_out[:], h1b[:])
    nc.compile()
    return nc


def phase1_inputs(x, W1, b1):
    """Per-core input maps for phase 1. x: [11, 8192] f32."""
    scale = W1_SCALE if PHASE1_MODE == "fp8dr" else 1.0
    npdt = NP_FP8 if PHASE1_MODE == "fp8dr" else NP_BF16

    # x image [128, NKT*11]: tiles 0..63 = k-major x^T, tile 64 = e vector
    # (partition 0 = 1, bias rider), tile 65 (fp8dr only) = zeros.
    xi = np.zeros((128, NKT, 11), np.float32)
    xi[:, 0:64, :] = x.T.reshape(64, 128, 11).transpose(1, 0, 2)
    xi[0, 64, :] = 1.0
    xi = xi.reshape(128, XCOLS).astype(npdt)

    maps = []
    for j in range(N_CORES):
        wj = np.zeros((128, NKT, 64), np.float32)
        w1j = scale * W1[:, 64 * j:64 * (j + 1)]          # [8192, 64]
        wj[:, 0:64, :] = w1j.reshape(64, 128, 64).transpose(1, 0, 2)
        wj[0, 64, :] = scale * b1[64 * j:64 * (j + 1)]
        maps.append({
            "x_img": xi,
            "w_img": wj.reshape(128, WCOLS).astype(npdt),
        })
    return maps


# ---------------------------------------------------------------------------
# phase 2: layers 2..3 + loss tail, single core
# ---------------------------------------------------------------------------

if P2_FP8:
    # imgA (sync ring, fp8): h1t image [128,44] | W2 DoubleRow-ordered
    # [128,1024] (h-major: per h, per tile-pair, [W2(2p,h)|W2(2p+1,h)]),
    # pre-scaled by W2_SCALE (fp8 subnormal dodge; undone via b2*scale +
    # W3/scale).
    A_DT = FP8
    _A_SHAPES = [("h1t", 128, 44), ("w2dr", 128, 1024)]
    _B_SHAPES = [
        ("w3", 128, 256), ("wi1a", 128, 64), ("wi1b7", 7, 64),
        ("mm6e", 7, 1), ("wi2e", 65, 32), ("wi3e", 33, 1),
    ]
else:
    A_DT = BF16
    _A_SHAPES = [
        ("h1t", 128, 44), ("w2a", 128, 512), ("wi1a", 128, 64),
        ("wi1b7", 7, 64), ("mm6e", 7, 1),
    ]
    _B_SHAPES = [
        ("w2b", 128, 512), ("w3", 128, 256), ("wi2e", 65, 32),
        ("wi3e", 33, 1),
    ]
A_OFF = {}
_c = 0
for _n, _p, _f in _A_SHAPES:
    A_OFF[_n] = _c
    _c += _f
IMGA_COLS = _c
B_OFF = {}
_c = 0
for _n, _p, _f in _B_SHAPES:
    B_OFF[_n] = _c
    _c += _f
IMGB_COLS = _c
# imgC (f32 consts, sync ring first): b2 [128,2] | b3c [128,1] |
# Imask [11,11] | wmean [11,1] | ones1 [1,11]
_C_SHAPES = [
    ("b2", 128, 2), ("b3c", 128, 1), ("imask", 11, 11), ("wmean", 11, 1),
    ("ones1", 1, 11),
]
C_OFF = {}
_c = 0
for _n, _p, _f in _C_SHAPES:
    C_OFF[_n] = _c
    _c += _f
IMGC_COLS = _c


def build_phase2():
    nc = bacc.Bacc("TRN2", target_bir_lowering=False, debug=False, num_devices=1)

    imgA = nc.dram_tensor("imgA", [128, IMGA_COLS], A_DT, kind="ExternalInput")
    imgB = nc.dram_tensor("imgB", [128, IMGB_COLS], BF16, kind="ExternalInput")
    imgC = nc.dram_tensor("imgC", [128, IMGC_COLS], F32, kind="ExternalInput")
    out = nc.dram_tensor("out", [1, 1], F32, kind="ExternalOutput")

    with tile.TileContext(nc) as tc:
        with (
            tc.tile_pool(name="sbuf", bufs=1) as sb,
            tc.tile_pool(name="psum", bufs=1, space="PSUM") as ps,
        ):
            # two DMAs per ring, split at matmul-consumption boundaries so
            # the first layer-2 matmuls start as soon as possible
            asb = sb.tile([128, IMGA_COLS], A_DT, tag="asb")
            bsb = sb.tile([128, IMGB_COLS], BF16, tag="bsb")
            ASPLIT = 44 + 512
            if P2_FP8:
                # fine split in consumption order: W2 h0-pair0 gates the
                # first matmul (small first chunks on every ring)
                nc.sync.dma_start(asb[:, 0:300], imgA[:, 0:300])
                nc.scalar.dma_start(bsb[:], imgB[:])
                nc.gpsimd.dma_start(asb[:, 300:ASPLIT], imgA[:, 300:ASPLIT])
                nc.sync.dma_start(asb[:, ASPLIT:ASPLIT + 256],
                                  imgA[:, ASPLIT:ASPLIT + 256])
                nc.scalar.dma_start(asb[:, ASPLIT + 256:IMGA_COLS],
                                    imgA[:, ASPLIT + 256:IMGA_COLS])
            else:
                nc.sync.dma_start(asb[:, 0:300], imgA[:, 0:300])
                nc.scalar.dma_start(bsb[:, 0:256], imgB[:, 0:256])
                nc.sync.dma_start(asb[:, 300:IMGA_COLS], imgA[:, 300:IMGA_COLS])
                nc.scalar.dma_start(bsb[:, 256:IMGB_COLS],
                                    imgB[:, 256:IMGB_COLS])
            csb = sb.tile([128, IMGC_COLS], F32, tag="csb")
            nc.gpsimd.dma_start(csb[:], imgC[:])

            def cs(name, p_, f_):
                c0 = C_OFF[name]
                return csb[0:p_, c0:c0 + f_]

            def bs(name, p_, f_):
                c0 = B_OFF[name]
                return bsb[0:p_, c0:c0 + f_]

            def as_(name, p_, f_):
                c0 = A_OFF[name]
                return asb[0:p_, c0:c0 + f_]

            b2sb = cs("b2", 128, 2)
            b3c = cs("b3c", 128, 1)
            imask = cs("imask", 11, 11)
            wmsb = cs("wmean", 11, 1)
            onsb = cs("ones1", 1, 11)

            w3sb = bs("w3", 128, 256)
            if P2_FP8:
                wi1asb = bs("wi1a", 128, 64)
                wi1b7 = bs("wi1b7", 7, 64)
                mm6e = bs("mm6e", 7, 1)
            else:
                wi1asb = as_("wi1a", 128, 64)
                wi1b7 = as_("wi1b7", 7, 64)
                mm6e = as_("mm6e", 7, 1)
            wi2e = bs("wi2e", 65, 32)
            wi3e = bs("wi3e", 33, 1)

            h1sb = asb[:, 0:44]

            def w2l(t, h):  # lhsT [128,128]: W2[128t+p, 128h+m]
                if t < 2:
                    return asb[:, 44 + 256 * t + 128 * h:44 + 256 * t + 128 * (h + 1)]
                return bsb[:, 256 * (t - 2) + 128 * h:256 * (t - 2) + 128 * (h + 1)]

            # hidden vectors with trailing 1.0 partition (bias K-extension)
            i1r = sb.tile([65, 1], BF16, tag="i1r")
            nc.vector.memset(i1r[64:65, :], 1.0)
            i2r = sb.tile([33, 1], BF16, tag="i2r")
            nc.vector.memset(i2r[32:33, :], 1.0)

            # exp activation-table prefetch, GpSimd extended-ucode library
            # prefetch (for the mid-chain partition_broadcast), and PE
            # warm-up -- all overlapped with the input DMA wait
            junk = sb.tile([128, 16], BF16, tag="junk")
            nc.vector.memset(junk[:], 0.0)
            jf = sb.tile([1, 2], F32, tag="jf")
            nc.vector.memset(jf[:], 1.0)
            ones128 = sb.tile([128, 11], F32, tag="ones128")
            nc.vector.memset(ones128[:], 1.0)
            nc.scalar.activation(jf[0:1, 1:2], jf[0:1, 0:1], AF.Exp)
            jps = ps.tile([11, 1], F32, tag="jps")
            for _ in range(12):
                nc.tensor.matmul(jps[:, :], junk[:, 0:11], junk[:, 0:1],
                                 start=True, stop=True)

            # ---- layer 2 direct to h2T: psum [128,11] per half h
            # (fp8 path: DoubleRow over k-tile pairs, W2 pre-scaled by
            # W2_SCALE; h2t carries the scale, undone in W3)
            h2t = sb.tile([128, 22], BF16, tag="h2t")
            for h in range(2):
                p2t = ps.tile([128, 11], F32, tag="pA", bufs=3)
                if P2_FP8:
                    DR2 = mybir.MatmulPerfMode.DoubleRow
                    for p in range(2):
                        c0 = 44 + 512 * h + 256 * p
                        nc.tensor.matmul(
                            p2t[:, :],
                            asb[:, c0:c0 + 256].rearrange(
                                "q (two f) -> q two f", two=2),
                            h1sb[:, 22 * p:22 * (p + 1)].rearrange(
                                "q (two f) -> q two f", two=2),
                            start=(p == 0), stop=(p == 1),
                            perf_mode=DR2,
                        )
                else:
                    for t in range(4):
                        nc.tensor.matmul(
                            p2t[:, :], w2l(t, h), h1sb[:, 11 * t:11 * (t + 1)],
                            start=(t == 0), stop=(t == 3),
                        )
                # relu(x + b2') on DVE, cast to bf16 (b2' pre-scaled host-side)
                nc.vector.tensor_scalar(h2t[:, 11 * h:11 * (h + 1)], p2t[:],
                                        b2sb[:, h:h + 1], 0.0, ALU.add, ALU.max)

            # ---- layer 3 col form: E^T [128, 11] (all 11 encodings)
            psET = ps.tile([128, 11], F32, tag="pA", bufs=3)
            for h in range(2):
                nc.tensor.matmul(
                    psET[:, :], w3sb[:, 128 * h:128 * (h + 1)],
                    h2t[:, 11 * h:11 * (h + 1)],
                    start=(h == 0), stop=(h == 1),
                )
            ETsb = sb.tile([128, 11], F32, tag="ETsb")
            # ET = psET + b3 (per-partition bias broadcast along free dim)
            nc.vector.tensor_scalar(ETsb[:], psET[:], b3c, 0.0,
                                    ALU.add, ALU.bypass)
            # E0 replicated along the free dim (feeds the G00 broadcast
            # matmul, which then overlaps the gram + diag)
            E0rep = sb.tile([128, 11], F32, tag="E0rep")
            nc.vector.tensor_scalar(E0rep[:], ones128, ETsb[0:128, 0:1], 0.0,
                                    ALU.mult, ALU.bypass)
            newT16 = sb.tile([128, 1], BF16, tag="newT16")
            nc.vector.tensor_copy(newT16[:], ETsb[:, 0:1])

            # ---- Gram matrix G = ET.T @ ET -> [11, 11] (f32 matmul)
            G = ps.tile([11, 11], F32, tag="pG")
            nc.tensor.matmul(G[:, :], ETsb[:], ETsb[:], start=True, stop=True)
            # ---- G00 = |E0|^2 on all 11 partitions, without waiting for
            # the diag: bc00 = E0rep.T @ E0
            bc00 = ps.tile([11, 1], F32, tag="pB")
            nc.tensor.matmul(bc00[:, :], E0rep[:], ETsb[:, 0:1],
                             start=True, stop=True)

            # ---- integrator MLP on [E_0; math_metrics] (parallel branch)
            i1c = ps.tile([64, 1], F32, tag="pC", bufs=2)
            nc.tensor.matmul(i1c[:, :], wi1asb, newT16[:, 0:1],
                             start=True, stop=False)
            nc.tensor.matmul(i1c[:, :], wi1b7, mm6e, start=False, stop=True)
            # integrator relus on the Scalar engine (relu is in every act
            # table set -> no extra table load; keeps the DVE chain clear)
            nc.scalar.activation(i1r[0:64, :], i1c[:, :], AF.Relu)

            # ---- diag(G) -> n2 [11,1] (DVE); col 0 of G = dvec (in place)
            scrG = sb.tile([11, 11], F32, tag="scrG")
            n2 = sb.tile([11, 1], F32, tag="n2")
            nc.vector.scalar_tensor_tensor(
                out=scrG[:], in0=G[:], scalar=1.0, in1=imask,
                op0=ALU.mult, op1=ALU.mult, accum_out=n2[:])
            dvec = G[:, 0:1]

            i2c = ps.tile([32, 1], F32, tag="pC", bufs=2)
            nc.tensor.matmul(i2c[:, :], wi2e, i1r[:, 0:1], start=True, stop=True)
            nc.scalar.activation(i2r[0:32, :], i2c[:, :], AF.Relu)

            # ---- ge [11,2]: col0 = clamp(g2), col1 = G00*n2
            ge = sb.tile([11, 2], F32, tag="ge")
            s12 = sb.tile([11, 1], F32, tag="s12")
            nc.vector.tensor_add(s12[:], n2[:], bc00[:])
            nc.vector.scalar_tensor_tensor(
                out=ge[:, 0:1], in0=dvec, scalar=-2.0, in1=s12[:],
                op0=ALU.mult, op1=ALU.add)
            nc.vector.tensor_mul(ge[:, 1:2], n2[:], bc00[:])
            nc.vector.tensor_scalar_max(ge[:, 0:1], ge[:, 0:1], 1e-12)

            qp = ps.tile([1, 1], F32, tag="pC", bufs=2)
            nc.tensor.matmul(qp[:, :], wi3e, i2r[:, 0:1], start=True, stop=True)
            il = sb.tile([1, 1], F32, tag="il")
            nc.scalar.activation(il[:], qp[:], AF.Exp, scale=-1.0)

            # ---- r = rsqrt([g2 | G00*n2]): quake seed + fused Newton
            # iters, DVE only (no second activation-table load).
            # geo = g2 * rsqrt(g2); rden = rsqrt(G00*n2) = 1/(|E0||Ej|).
            I32 = mybir.dt.int32
            seedb = sb.tile([11, 2], I32, tag="seedb")
            nc.vector.tensor_scalar(seedb[:], ge[:].bitcast(I32), 1, None,
                                    ALU.arith_shift_right)
            # r0 = bitcast(0x5f3759df - (bits >> 1))
            nc.vector.tensor_scalar(seedb[:], seedb[:], -1, 0x5F3759DF,
                                    ALU.mult, ALU.add)
            r = sb.tile([11, 2], F32, tag="rsq")
            rr = sb.tile([11, 2], F32, tag="rr")
            rcur = seedb[:].bitcast(F32)
            for _ in range(NEWTON_ITERS):
                nc.vector.tensor_mul(rr[:], rcur, rcur)
                # t2 = (-0.5*r^2) * ge
                nc.vector.scalar_tensor_tensor(
                    out=rr[:], in0=rr[:], scalar=-0.5, in1=ge[:],
                    op0=ALU.mult, op1=ALU.mult)
                # r' = (t2 + 1.5) * r
                nc.vector.scalar_tensor_tensor(
                    out=r[:], in0=rr[:], scalar=1.5, in1=rcur,
                    op0=ALU.add, op1=ALU.mult)
                rcur = r[:]
            geo = sb.tile([11, 1], F32, tag="geo")
            nc.vector.tensor_mul(geo[:], ge[:, 0:1], r[:, 0:1])

            # negscore = cos - geo = dvec*rden - geo
            negscore = sb.tile([11, 1], F32, tag="negscore")
            nc.vector.scalar_tensor_tensor(
                out=negscore[:], in0=dvec, scalar=r[0:11, 1:2],
                in1=geo[:], op0=ALU.mult, op1=ALU.subtract)

            # mean over rows 1..10 with negated weights (wmean = [0, -0.1 x10])
            meanp = ps.tile([1, 1], F32, tag="pC", bufs=2)
            nc.tensor.matmul(meanp[:, :], negscore[:, 0:1], wmsb,
                             start=True, stop=True)

            total = sb.tile([1, 1], F32, tag="total")
            nc.vector.tensor_add(total[:], il[:], meanp[:])
            nc.sync.dma_start(out[:], total[:], single_packet=True)
    nc.compile()
    return nc


def phase2_inputs(h1t_full, W2, b2, W3, b3, Wi1, bi1, Wi2, bi2, Wi3, bi3,
                  math_metrics):
    """h1t_full: [512, 11] f32-ish = concat of per-core [64, 11] outputs."""
    ints = {
        "wi1a": Wi1[:128],
        "wi1b7": np.concatenate([Wi1[128:], bi1.reshape(1, 64)], axis=0),
        "mm6e": np.concatenate([math_metrics.reshape(6, 1),
                                np.ones((1, 1), np.float32)], axis=0),
        "wi2e": np.concatenate([Wi2, bi2.reshape(1, 32)], axis=0),
        "wi3e": np.concatenate([Wi3, bi3.reshape(1, 1)], axis=0),
    }
    if P2_FP8:
        # DoubleRow-ordered W2, pre-scaled; scale undone via W3/scale and
        # pre-scaled b2 (h2t carries the scale)
        w2s = (W2_SCALE * W2).astype(np.float32)
        w2dr = np.zeros((128, 1024), np.float32)
        for h in range(2):
            for pair in range(2):
                for sub in range(2):
                    c = 512 * h + 256 * pair + 128 * sub
                    kt = 2 * pair + sub
                    w2dr[:, c:c + 128] = w2s[128 * kt:128 * (kt + 1),
                                             128 * h:128 * (h + 1)]
        avals = {"h1t": _kmajor_image(h1t_full.astype(np.float32)),
                 "w2dr": w2dr}
        bvals = {"w3": _kmajor_image(W3) / W2_SCALE, **ints}
        b2c = (W2_SCALE * b2).reshape(2, 128).T
        a_np = NP_FP8
    else:
        w2img = _kmajor_image(W2.astype(np.float32))  # [128, 1024]
        avals = {
            "h1t": _kmajor_image(h1t_full.astype(np.float32)),
            "w2a": w2img[:, 0:512],
            "wi1a": ints["wi1a"], "wi1b7": ints["wi1b7"],
            "mm6e": ints["mm6e"],
        }
        bvals = {
            "w2b": w2img[:, 512:1024],
            "w3": _kmajor_image(W3),
            "wi2e": ints["wi2e"], "wi3e": ints["wi3e"],
        }
        b2c = b2.reshape(2, 128).T
        a_np = NP_BF16

    imgA = np.zeros((128, IMGA_COLS), np.float32)
    for name, p, f in _A_SHAPES:
        v = np.asarray(avals[name], np.float32)
        assert v.shape == (p, f), (name, v.shape, (p, f))
        imgA[:p, A_OFF[name]:A_OFF[name] + f] = v

    imgB = np.zeros((128, IMGB_COLS), np.float32)
    for name, p, f in _B_SHAPES:
        v = np.asarray(bvals[name], np.float32)
        assert v.shape == (p, f), (name, v.shape, (p, f))
        imgB[:p, B_OFF[name]:B_OFF[name] + f] = v

    wm = np.zeros((11, 1), np.float32)
    wm[1:, 0] = -0.1
    cvals = {
        "b2": b2c,
        "b3c": b3.reshape(128, 1),
        "imask": np.eye(11, dtype=np.float32),
        "wmean": wm,
        "ones1": np.ones((1, 11), np.float32),
    }
    imgC = np.zeros((128, IMGC_COLS), np.float32)
    for name, p, f in _C_SHAPES:
        v = np.asarray(cvals[name], np.float32)
        assert v.shape == (p, f), (name, v.shape, (p, f))
        imgC[:p, C_OFF[name]:C_OFF[name] + f] = v
    return {"imgA": imgA.astype(a_np), "imgB": imgB.astype(NP_BF16),
            "imgC": imgC}


# ---------------------------------------------------------------------------
# entry point
# ---------------------------------------------------------------------------

_NC1 = None
_NC2 = None


def _get_ncs():
    global _NC1, _NC2
    if _NC1 is None:
        _NC1 = build_phase1()
        _NC2 = build_phase2()
    return _NC1, _NC2


def kernel(new_knowledge, existing_knowledge, math_metrics,
           W1, b1, W2, b2, W3, b3, Wi1, bi1, Wi2, bi2, Wi3, bi3):
    args = [new_knowledge, existing_knowledge, math_metrics,
            W1, b1, W2, b2, W3, b3, Wi1, bi1, Wi2, bi2, Wi3, bi3]
    (new_knowledge, existing_knowledge, math_metrics,
     W1, b1, W2, b2, W3, b3, Wi1, bi1, Wi2, bi2, Wi3, bi3) = [
        np.asarray(a, np.float32) for a in args]

    nc1, nc2 = _get_ncs()

    x = np.concatenate([new_knowledge[None, :], existing_knowledge], axis=0)
    maps1 = phase1_inputs(x, W1, b1)
    res1 = bass_utils.run_bass_kernel_spmd(
        nc1, maps1, core_ids=list(range(N_CORES)))
    # pure gather: concat per-core transposed h1 blocks -> [512, 11]
    h1t_full = np.concatenate(
        [np.asarray(res1.results[j]["h1t_out"], np.float32)
         for j in range(N_CORES)], axis=0)

    maps2 = [phase2_inputs(h1t_full, W2, b2, W3, b3,
                           Wi1, bi1, Wi2, bi2, Wi3, bi3, math_metrics)]
    res2 = bass_utils.run_bass_kernel_spmd(nc2, maps2, core_ids=[0])
    return res2.results[0]["out"].reshape(()).astype(np.float32)
